# revision 17
# baseline (speedup 1.0000x reference)
"""GCN layer on 8 TRN2 NeuronCores (Bass/Tile).

out = segment_sum(edge_vals[:,None] * (X @ W)[edge_col], edge_row, N)

Strategy (1D destination-node sharding):
  - Host: cast/transpose X -> XT bf16 (replicated to all 8 cores). Partition
    edges by destination shard (6250 rows/core), group by destination window
    (128 rows), and within each window split by source THIRD of the permuted
    xw table so the int16 dma_gather indices stay in range AND so the three
    sub-streams can run on three SWDGE queues (Q7 core pairs 0-1 / 2-3 / 4-5)
    whose descriptor generation overlaps. Pad each (window, third) group to a
    multiple of 128 edges with zero-weight edges; tile counts are maxed
    across cores so all 8 cores run the identical (SPMD) program.
  - Device phase 1: XW = X @ W computed redundantly per core (TensorE bf16,
    fp32 PSUM), stored bf16 into a DRAM scratch with rows padded to 128 cols
    (256B - dma_gather's granularity) in a partition-major permuted order so
    the stores are a few large contiguous DMAs.
  - Device phase 2: dma_gather (SWDGE descriptor-per-edge, one queue per
    source third) fetches the XW rows for each 128-edge tile; VectorE builds
    S[e, r] = val[e] * (row_local[e] == r) via an iota compare; TensorE
    accumulates S^T @ rows into the window's PSUM [128, 64]. Windows are
    written out dense - no scatter races anywhere.
  - Host: concatenate the 8 output shards.

The Q7 SWDGE descriptor generation (~7.9 ns/edge) is the kernel's critical
path; splitting it across 3 queues runs it on 3 Q7 core pairs concurrently.
"""

from contextlib import ExitStack

import ml_dtypes
import numpy as np

import concourse.bacc as bacc
import concourse.bass as bass
import concourse.mybir as mybir
import concourse.tile as tile
from concourse._compat import get_trn_type
from concourse.bass_utils import run_bass_kernel_spmd

N_NODES = 50000
N_EDGES = 800000
F_IN = 256
F_OUT = 64
N_CORES = 8
SHARD = N_NODES // N_CORES  # 6250 destination rows per core
WIN = 128  # destination rows per PSUM accumulation window
BF16 = ml_dtypes.bfloat16
NSPLIT = 3  # source thirds == SWDGE queues (queue 3 crashes the device)

# knobs
SLAB = 4096  # phase-1 node columns per XT slab DMA
GRP = 32  # phase-1 node tiles per staged XW store DMA
CH = 32  # phase-2 edge tiles (of 128 edges) per dma_gather call
GB = 8  # phase-2 edge tiles per batched one-hot / rhs-scale (divides CH)
SIM_MEMSET = False  # zero staging tiles (only needed to appease CoreSim)

# test.py pokes these for profiling
TRACE = False
LAST_RESULTS = None


def _install_ntff_hook():
    """The agent image's antenv lacks axon_hooks, so bass_utils' trace=True
    path can't find the NTFF hook. Recreate the module and register the
    ctypes-based hook exactly as trn_agent_boot would."""
    import sys
    import types

    try:
        import antenv.axon_hooks  # noqa: F401

        return True
    except ImportError:
        pass
    try:
        import antenv
        from trn_agent_boot.trn_boot import _ntff_profile_via_ctypes

        mod = types.ModuleType("antenv.axon_hooks")
        mod._hook = None

        def set_axon_ntff_profile_hook(h):
            mod._hook = h

        def get_axon_ntff_profile_hook():
            return mod._hook

        mod.set_axon_ntff_profile_hook = set_axon_ntff_profile_hook
        mod.get_axon_ntff_profile_hook = get_axon_ntff_profile_hook
        sys.modules["antenv.axon_hooks"] = mod
        antenv.axon_hooks = mod
        hook = _ntff_profile_via_ctypes("/opt/axon/libaxon_pjrt.so")
        if hook is not None:
            set_axon_ntff_profile_hook(hook)
        return hook is not None
    except Exception as e:  # profiling is best-effort
        print(f"ntff hook install failed: {e}")
        return False


def _wrap16(stream_i16, n_tiles):
    """Wrapped+replicated dma_gather index layout: stream position i lives at
    partition i%16 (replicated to all 8 16-partition groups), slot i//16."""
    n = n_tiles * 128
    w = np.zeros((128, n // 16), dtype=np.int16)
    s = np.zeros(n, dtype=np.int16)
    s[: len(stream_i16)] = stream_i16
    blk = s.reshape(n // 16, 16).T  # [16, n//16]
    for g in range(8):
        w[g * 16 : (g + 1) * 16, :] = blk
    return w


def _split_rows(nsplit):
    """Row ranges of the permuted xw table for each source split."""
    NT = (N_NODES + 127) // 128
    total = 128 * NT
    step = -(-total // nsplit)
    return [(s * step, min((s + 1) * step, total)) for s in range(nsplit)]


def _prep(X, W, edge_row, edge_col, edge_vals):
    """Host-side sharding/marshalling.

    Returns (in_maps, T): T[s] is the per-window tile count of split s
    (identical across cores -> shared SPMD program).
    """
    XT = np.ascontiguousarray(X.T).astype(BF16)  # [F_IN, N_NODES]
    Wb = np.ascontiguousarray(W).astype(BF16)  # [F_IN, F_OUT]
    iota = np.tile(np.arange(WIN, dtype=np.float32), (128, GB))  # [128, GB*WIN]

    NT = (N_NODES + 127) // 128  # 391 node tiles; xw virtual rows = 128*NT
    ranges = _split_rows(NSPLIT)

    n_win = (SHARD + WIN - 1) // WIN
    core = edge_row // SHARD
    percore = []
    cnt = np.zeros((NSPLIT, N_CORES, n_win), dtype=np.int64)
    for p in range(N_CORES):
        m = core == p
        r = edge_row[m].astype(np.int64) - p * SHARD
        c = edge_col[m].astype(np.int64)
        v = edge_vals[m].astype(np.float32)
        q = (c % 128) * NT + c // 128  # permuted xw virtual row
        sp = np.minimum(q // ranges[0][1], NSPLIT - 1)
        w = r // WIN
        # order: (window, split) groups; stable within
        order = np.lexsort((sp, w))
        r, q, v, sp, w = r[order], q[order], v[order], sp[order], w[order]
        percore.append((r, q, v, sp, w))
        for wi in range(n_win):
            mw = w == wi
            for s in range(NSPLIT):
                cnt[s, p, wi] = (mw & (sp == s)).sum()

    T = [np.maximum(1, -(-cnt[s].max(axis=0) // 128)) for s in range(NSPLIT)]
    Js = [int(t.sum()) for t in T]
    J = sum(Js)
    starts = [np.concatenate([[0], np.cumsum(t)]) for t in T]

    in_maps = []
    for p in range(N_CORES):
        r, q, v, sp, w = percore[p]
        qbufs = [np.zeros(Js[s] * 128, dtype=np.int64) for s in range(NSPLIT)]
        # consumption-order meta: per window, T[0][w] split-0 tiles, then
        # T[1][w] split-1 tiles, ...
        vals = np.zeros(J * 128, dtype=np.float32)
        rowloc = np.zeros(J * 128, dtype=np.float32)
        for wi in range(n_win):
            for s in range(NSPLIT):
                mw = (w == wi) & (sp == s)
                n = int(mw.sum())
                s0 = int(starts[s][wi]) * 128
                qbufs[s][s0 : s0 + n] = q[mw] - ranges[s][0]
                # meta position: window base + tiles of earlier splits
                mb = (
                    sum(int(starts[s2][wi]) for s2 in range(NSPLIT))
                    + sum(int(T[s2][wi]) for s2 in range(s))
                ) * 128
                vals[mb : mb + n] = v[mw]
                rowloc[mb : mb + n] = (r[mw] % WIN).astype(np.float32)
        meta = np.concatenate(
            [rowloc.reshape(J, 128).T, vals.reshape(J, 128).T, iota], axis=1
        )
        im = {"xt": XT, "w": Wb, "meta": np.ascontiguousarray(meta)}
        for s in range(NSPLIT):
            im[f"cols{s}"] = _wrap16(qbufs[s].astype(np.int16), Js[s])
        in_maps.append(im)
    return in_maps, T


def _build_nc(T, n_nodes=N_NODES, f_in=F_IN, f_out=F_OUT, shard=SHARD):
    f32 = mybir.dt.float32
    bf16 = mybir.dt.bfloat16
    i16 = mybir.dt.int16
    n_win = len(T[0])
    Js = [int(t.sum()) for t in T]
    J = sum(Js)
    starts = [np.concatenate([[0], np.cumsum(t)]) for t in T]

    NT = (n_nodes + 127) // 128  # node tiles
    ranges = _split_rows(NSPLIT)

    nc = bacc.Bacc(
        get_trn_type() or "TRN2",
        target_bir_lowering=False,
        dynamic_dma_scratch_size=32768,
        num_swdge_queues=NSPLIT,
    )
    xt = nc.dram_tensor("xt", [f_in, n_nodes], bf16, kind="ExternalInput")
    w_in = nc.dram_tensor("w", [f_in, f_out], bf16, kind="ExternalInput")
    cols_d = [
        nc.dram_tensor(f"cols{s}", [128, Js[s] * 8], i16, kind="ExternalInput")
        for s in range(NSPLIT)
    ]
    meta = nc.dram_tensor("meta", [128, 2 * J + GB * WIN], f32, kind="ExternalInput")
    out = nc.dram_tensor("out", [shard, f_out], f32, kind="ExternalOutput")
    # XW scratch: virtual row p*NT + t = XW[t*128 + p], cols padded 64->128
    # so each row is 256B (dma_gather granularity).
    xw = nc.dram_tensor("xw", [128 * NT, 128], bf16, kind="Internal")

    n_kc = f_in // 128  # contraction chunks (2)

    with tile.TileContext(nc) as tc, ExitStack() as ctx:
        const = ctx.enter_context(tc.tile_pool(name="const", bufs=1))
        xt_pool = ctx.enter_context(tc.tile_pool(name="xtp", bufs=2))
        psum1 = ctx.enter_context(tc.tile_pool(name="psum1", bufs=6, space="PSUM"))
        xw_sb = ctx.enter_context(tc.tile_pool(name="xw_sb", bufs=2))
        gath = ctx.enter_context(tc.tile_pool(name="gath", bufs=2))
        s_pool = ctx.enter_context(tc.tile_pool(name="s_pool", bufs=4))
        rhs_pool = ctx.enter_context(tc.tile_pool(name="rhs_pool", bufs=4))
        psum2 = ctx.enter_context(tc.tile_pool(name="psum2", bufs=2, space="PSUM"))
        out_sb = ctx.enter_context(tc.tile_pool(name="out_sb", bufs=4))

        # resident constants
        w_t = []
        for k in range(n_kc):
            wt = const.tile([128, f_out], bf16, tag=f"w{k}")
            nc.sync.dma_start(out=wt[:], in_=w_in[k * 128 : (k + 1) * 128, :])
            w_t.append(wt)
        meta_t = const.tile([128, 2 * J + GB * WIN], f32, tag="meta")
        nc.sync.dma_start(out=meta_t[:], in_=meta[:, :])
        cols_t = []
        for s in range(NSPLIT):
            ct = const.tile([128, Js[s] * 8], i16, tag=f"c{s}")
            nc.sync.dma_start(out=ct[:], in_=cols_d[s][:, :])
            cols_t.append(ct)

        # ---- phase 1: xw = (X @ W) in bf16, partition-major, 128-padded ----
        xw_pm = xw[:, :].rearrange("(p t) f -> p (t f)", p=128)  # [128, NT*128]
        stg = None
        g0 = 0
        s0 = 0
        xts = []
        for nt_i in range(NT):
            n0 = nt_i * 128
            m = min(128, n_nodes - n0)
            if nt_i % (SLAB // 128) == 0:
                s0 = n0
                sl = min(SLAB, n_nodes - s0)
                xts = []
                for k in range(n_kc):
                    xtk = xt_pool.tile([128, SLAB], bf16, tag=f"xt{k}")
                    nc.sync.dma_start(
                        out=xtk[:, :sl],
                        in_=xt[k * 128 : (k + 1) * 128, s0 : s0 + sl],
                    )
                    xts.append(xtk)
            if nt_i % GRP == 0:
                g0 = nt_i
                stg = xw_sb.tile([128, GRP * 128], bf16, tag="stg")
                if SIM_MEMSET:  # garbage bytes are never consumed on HW
                    nc.gpsimd.memset(stg[:], 0)
            ps = psum1.tile([128, f_out], f32, tag="ps1")
            for k in range(n_kc):
                nc.tensor.matmul(
                    out=ps[:m, :],
                    lhsT=xts[k][:, n0 - s0 : n0 - s0 + m],
                    rhs=w_t[k][:],
                    start=(k == 0),
                    stop=(k == n_kc - 1),
                )
            loc = nt_i - g0
            nc.vector.tensor_copy(
                out=stg[:m, loc * 128 : loc * 128 + f_out], in_=ps[:m, :]
            )
            if nt_i == NT - 1 or (nt_i + 1) % GRP == 0:
                gn = nt_i + 1 - g0
                nc.sync.dma_start(
                    out=xw_pm[:, g0 * 128 : (g0 + gn) * 128],
                    in_=stg[:, : gn * 128],
                )

        # ---- phase 2: dma_gather + batched one-hot matmul segment-sum ----
        # meta column of a (window, split, k) tile in consumption order
        def meta_col(w, s, k):
            return (
                sum(int(starts[s2][w]) for s2 in range(NSPLIT))
                + sum(int(T[s2][w]) for s2 in range(s))
                + k
            )

        # stream-position -> meta column (needed for batched builds)
        m_of = [np.zeros(Js[s], dtype=np.int64) for s in range(NSPLIT)]
        for w in range(n_win):
            for s in range(NSPLIT):
                for k in range(int(T[s][w])):
                    m_of[s][int(starts[s][w]) + k] = meta_col(w, s, k)

        chunks = {}  # (s, chunk_idx) -> gather tile
        batches = {}  # (s, batch_idx) -> (S_b, rhs_b)

        def ensure_chunk(s, tile_idx):
            ci = tile_idx // CH
            key = (s, ci)
            if key in chunks:
                return chunks[key]
            lo, hi_r = ranges[s]
            cn = min(CH, Js[s] - ci * CH)
            g = gath.tile([128, CH, 128], bf16, tag=f"g{s}")
            nc.gpsimd.dma_gather(
                out_ap=g[:, :cn, :],
                in_ap=xw[lo:hi_r, :],
                idxs_ap=cols_t[s][:, ci * CH * 8 : (ci * CH + cn) * 8],
                num_idxs=cn * 128,
                num_idxs_reg=cn * 128,
                elem_size=128,
                single_packet=False,
                # one SWDGE queue (= Q7 core pair) per source split: the
                # three desc-gen streams overlap.
                queue_num=s,
            )
            chunks[key] = g
            return g

        def ensure_batch(s, tile_idx):
            bi = tile_idx // GB
            key = (s, bi)
            if key in batches:
                return batches[key]
            b0 = bi * GB
            bn = min(GB, Js[s] - b0)
            g = ensure_chunk(s, b0)
            gs = b0 - (b0 // CH) * CH  # batch offset within its chunk
            # meta columns of this batch are contiguous within a (window,
            # split) group but the batch may span groups; split into
            # contiguous runs.
            cols = m_of[s][b0 : b0 + bn]
            S_b = s_pool.tile([128, GB, WIN], bf16, tag=f"S{s}")
            rhs_b = rhs_pool.tile([128, GB, f_out], bf16, tag=f"r{s}")
            runs = []
            r0 = 0
            for i in range(1, bn + 1):
                if i == bn or cols[i] != cols[i - 1] + 1:
                    runs.append((r0, i))
                    r0 = i
            for a, b in runs:
                n = b - a
                mc = int(cols[a])
                nc.vector.tensor_tensor(
                    out=S_b[:, a:b, :],
                    in0=meta_t[:, 2 * J : 2 * J + n * WIN].rearrange(
                        "p (b r) -> p b r", r=WIN
                    ),
                    in1=meta_t[:, mc : mc + n].to_broadcast([128, n, WIN]),
                    op=mybir.AluOpType.is_equal,
                )
                nc.vector.tensor_tensor(
                    out=rhs_b[:, a:b, :],
                    in0=g[:, gs + a : gs + b, 0:f_out],
                    in1=meta_t[:, J + mc : J + mc + n].to_broadcast(
                        [128, n, f_out]
                    ),
                    op=mybir.AluOpType.mult,
                )
            batches[key] = (S_b, rhs_b)
            return batches[key]

        for w in range(n_win):
            cur_ps = psum2.tile([128, f_out], f32, tag="ps2")
            n_t = int(sum(T[s][w] for s in range(NSPLIT)))
            ti = 0  # tile index within this window's consumption order
            for s in range(NSPLIT):
                for k in range(int(T[s][w])):
                    t_s = int(starts[s][w]) + k  # stream position
                    S_b, rhs_b = ensure_batch(s, t_s)
                    sl = t_s % GB
                    nc.tensor.matmul(
                        out=cur_ps[:],
                        lhsT=S_b[:, sl : sl + 1, :],
                        rhs=rhs_b[:, sl : sl + 1, :],
                        start=(ti == 0),
                        stop=(ti == n_t - 1),
                    )
                    ti += 1
            rows = min(WIN, shard - w * WIN)
            ot = out_sb.tile([128, f_out], f32, tag="ot")
            nc.vector.tensor_copy(out=ot[:rows, :], in_=cur_ps[:rows, :])
            nc.sync.dma_start(out=out[w * WIN : w * WIN + rows, :], in_=ot[:rows, :])
    nc.compile()
    return nc


def kernel(X, W, edge_row, edge_col, edge_vals):
    global LAST_RESULTS
    X = np.asarray(X, dtype=np.float32)
    W = np.asarray(W, dtype=np.float32)
    edge_row = np.asarray(edge_row, dtype=np.int32)
    edge_col = np.asarray(edge_col, dtype=np.int32)
    edge_vals = np.asarray(edge_vals, dtype=np.float32)

    in_maps, T = _prep(X, W, edge_row, edge_col, edge_vals)
    nc = _build_nc(T)
    trace = TRACE and _install_ntff_hook()
    res = run_bass_kernel_spmd(
        nc, in_maps, core_ids=list(range(N_CORES)), trace=trace
    )
    LAST_RESULTS = res
    out = np.concatenate([res.results[p]["out"] for p in range(N_CORES)], axis=0)
    return out.astype(np.float32)


# revision 23
# speedup vs baseline: 1.0221x; 1.0221x over previous
"""GCN layer on 8 TRN2 NeuronCores (Bass/Tile).

out = segment_sum(edge_vals[:,None] * (X @ W)[edge_col], edge_row, N)

Strategy (1D destination-node sharding):
  - Host: cast/transpose X -> XT bf16 (replicated to all 8 cores). Partition
    edges by destination shard (6250 rows/core), group by destination window
    (128 rows), and within each window split by source THIRD of the permuted
    xw table so the int16 dma_gather indices stay in range AND so the three
    sub-streams can run on three SWDGE queues (Q7 core pairs 0-1 / 2-3 / 4-5)
    whose descriptor generation overlaps. Pad each (window, third) group to a
    multiple of 128 edges with zero-weight edges; tile counts are maxed
    across cores so all 8 cores run the identical (SPMD) program.
  - Device phase 1: XW = X @ W computed redundantly per core (TensorE bf16,
    fp32 PSUM), stored bf16 into a DRAM scratch with rows padded to 128 cols
    (256B - dma_gather's granularity) in a partition-major permuted order so
    the stores are a few large contiguous DMAs.
  - Device phase 2: dma_gather (SWDGE descriptor-per-edge, one queue per
    source third) fetches the XW rows for each 128-edge tile; VectorE builds
    S[e, r] = val[e] * (row_local[e] == r) via an iota compare; TensorE
    accumulates S^T @ rows into the window's PSUM [128, 64]. Windows are
    written out dense - no scatter races anywhere.
  - Host: concatenate the 8 output shards.

The Q7 SWDGE descriptor generation (~7.9 ns/edge) is the kernel's critical
path; splitting it across 3 queues runs it on 3 Q7 core pairs concurrently.
"""

from contextlib import ExitStack

import ml_dtypes
import numpy as np

import concourse.bacc as bacc
import concourse.bass as bass
import concourse.mybir as mybir
import concourse.tile as tile
from concourse._compat import get_trn_type
from concourse.bass_utils import run_bass_kernel_spmd

N_NODES = 50000
N_EDGES = 800000
F_IN = 256
F_OUT = 64
N_CORES = 8
SHARD = N_NODES // N_CORES  # 6250 destination rows per core
WIN = 128  # destination rows per PSUM accumulation window
BF16 = ml_dtypes.bfloat16
NSPLIT = 3  # source thirds == SWDGE queues (queue 3 crashes the device)

# knobs
SLAB = 4096  # phase-1 node columns per XT slab DMA
GRP = 32  # phase-1 node tiles per staged XW store DMA
CH = 32  # phase-2 edge tiles (of 128 edges) per dma_gather call
GB = 8  # phase-2 edge tiles per batched one-hot / rhs-scale (divides CH)
SIM_MEMSET = False  # zero staging tiles (only needed to appease CoreSim)

# test.py pokes these for profiling
TRACE = False
LAST_RESULTS = None


def _install_ntff_hook():
    """The agent image's antenv lacks axon_hooks, so bass_utils' trace=True
    path can't find the NTFF hook. Recreate the module and register the
    ctypes-based hook exactly as trn_agent_boot would."""
    import sys
    import types

    try:
        import antenv.axon_hooks  # noqa: F401

        return True
    except ImportError:
        pass
    try:
        import antenv
        from trn_agent_boot.trn_boot import _ntff_profile_via_ctypes

        mod = types.ModuleType("antenv.axon_hooks")
        mod._hook = None

        def set_axon_ntff_profile_hook(h):
            mod._hook = h

        def get_axon_ntff_profile_hook():
            return mod._hook

        mod.set_axon_ntff_profile_hook = set_axon_ntff_profile_hook
        mod.get_axon_ntff_profile_hook = get_axon_ntff_profile_hook
        sys.modules["antenv.axon_hooks"] = mod
        antenv.axon_hooks = mod
        hook = _ntff_profile_via_ctypes("/opt/axon/libaxon_pjrt.so")
        if hook is not None:
            set_axon_ntff_profile_hook(hook)
        return hook is not None
    except Exception as e:  # profiling is best-effort
        print(f"ntff hook install failed: {e}")
        return False


def _wrap16(stream_i16, n_tiles):
    """Wrapped+replicated dma_gather index layout: stream position i lives at
    partition i%16 (replicated to all 8 16-partition groups), slot i//16."""
    n = n_tiles * 128
    w = np.zeros((128, n // 16), dtype=np.int16)
    s = np.zeros(n, dtype=np.int16)
    s[: len(stream_i16)] = stream_i16
    blk = s.reshape(n // 16, 16).T  # [16, n//16]
    for g in range(8):
        w[g * 16 : (g + 1) * 16, :] = blk
    return w


def _split_rows(nsplit):
    """Row ranges of the permuted xw table for each source split."""
    NT = (N_NODES + 127) // 128
    total = 128 * NT
    step = -(-total // nsplit)
    return [(s * step, min((s + 1) * step, total)) for s in range(nsplit)]


def _prep(X, W, edge_row, edge_col, edge_vals):
    """Host-side sharding/marshalling.

    Returns (in_maps, T): T[s] is the per-window tile count of split s
    (identical across cores -> shared SPMD program).
    """
    XT = np.ascontiguousarray(X.T).astype(BF16)  # [F_IN, N_NODES]
    Wb = np.ascontiguousarray(W).astype(BF16)  # [F_IN, F_OUT]
    iota = np.tile(np.arange(WIN, dtype=np.float32), (128, GB))  # [128, GB*WIN]

    NT = (N_NODES + 127) // 128  # 391 node tiles; xw virtual rows = 128*NT
    ranges = _split_rows(NSPLIT)

    n_win = (SHARD + WIN - 1) // WIN
    core = edge_row // SHARD
    percore = []
    cnt = np.zeros((NSPLIT, N_CORES, n_win), dtype=np.int64)
    for p in range(N_CORES):
        m = core == p
        r = edge_row[m].astype(np.int64) - p * SHARD
        c = edge_col[m].astype(np.int64)
        v = edge_vals[m].astype(np.float32)
        q = (c % 128) * NT + c // 128  # permuted xw virtual row
        sp = np.minimum(q // ranges[0][1], NSPLIT - 1)
        w = r // WIN
        # order: (window, split) groups; stable within
        order = np.lexsort((sp, w))
        r, q, v, sp, w = r[order], q[order], v[order], sp[order], w[order]
        percore.append((r, q, v, sp, w))
        for wi in range(n_win):
            mw = w == wi
            for s in range(NSPLIT):
                cnt[s, p, wi] = (mw & (sp == s)).sum()

    T = [np.maximum(1, -(-cnt[s].max(axis=0) // 128)) for s in range(NSPLIT)]
    Js = [int(t.sum()) for t in T]
    J = sum(Js)
    starts = [np.concatenate([[0], np.cumsum(t)]) for t in T]

    in_maps = []
    for p in range(N_CORES):
        r, q, v, sp, w = percore[p]
        qbufs = [np.zeros(Js[s] * 128, dtype=np.int64) for s in range(NSPLIT)]
        # consumption-order meta: per window, T[0][w] split-0 tiles, then
        # T[1][w] split-1 tiles, ...
        vals = np.zeros(J * 128, dtype=np.float32)
        rowloc = np.zeros(J * 128, dtype=np.float32)
        for wi in range(n_win):
            for s in range(NSPLIT):
                mw = (w == wi) & (sp == s)
                n = int(mw.sum())
                s0 = int(starts[s][wi]) * 128
                qbufs[s][s0 : s0 + n] = q[mw] - ranges[s][0]
                # meta position: window base + tiles of earlier splits
                mb = (
                    sum(int(starts[s2][wi]) for s2 in range(NSPLIT))
                    + sum(int(T[s2][wi]) for s2 in range(s))
                ) * 128
                vals[mb : mb + n] = v[mw]
                rowloc[mb : mb + n] = (r[mw] % WIN).astype(np.float32)
        meta = np.concatenate(
            [rowloc.reshape(J, 128).T, vals.reshape(J, 128).T, iota], axis=1
        )
        im = {"xt": XT, "w": Wb, "meta": np.ascontiguousarray(meta)}
        for s in range(NSPLIT):
            im[f"cols{s}"] = _wrap16(qbufs[s].astype(np.int16), Js[s])
        in_maps.append(im)
    return in_maps, T


def _build_nc(T, n_nodes=N_NODES, f_in=F_IN, f_out=F_OUT, shard=SHARD):
    f32 = mybir.dt.float32
    bf16 = mybir.dt.bfloat16
    i16 = mybir.dt.int16
    n_win = len(T[0])
    Js = [int(t.sum()) for t in T]
    J = sum(Js)
    starts = [np.concatenate([[0], np.cumsum(t)]) for t in T]

    NT = (n_nodes + 127) // 128  # node tiles
    ranges = _split_rows(NSPLIT)

    nc = bacc.Bacc(
        get_trn_type() or "TRN2",
        target_bir_lowering=False,
        dynamic_dma_scratch_size=32768,
        num_swdge_queues=NSPLIT,
    )
    xt = nc.dram_tensor("xt", [f_in, n_nodes], bf16, kind="ExternalInput")
    w_in = nc.dram_tensor("w", [f_in, f_out], bf16, kind="ExternalInput")
    cols_d = [
        nc.dram_tensor(f"cols{s}", [128, Js[s] * 8], i16, kind="ExternalInput")
        for s in range(NSPLIT)
    ]
    meta = nc.dram_tensor("meta", [128, 2 * J + GB * WIN], f32, kind="ExternalInput")
    out = nc.dram_tensor("out", [shard, f_out], f32, kind="ExternalOutput")
    # XW scratch: virtual row p*NT + t = XW[t*128 + p], cols padded 64->128
    # so each row is 256B (dma_gather granularity).
    xw = nc.dram_tensor("xw", [128 * NT, 128], bf16, kind="Internal")

    n_kc = f_in // 128  # contraction chunks (2)

    with tile.TileContext(nc) as tc, ExitStack() as ctx:
        const = ctx.enter_context(tc.tile_pool(name="const", bufs=1))
        xt_pool = ctx.enter_context(tc.tile_pool(name="xtp", bufs=2))
        psum1 = ctx.enter_context(tc.tile_pool(name="psum1", bufs=6, space="PSUM"))
        xw_sb = ctx.enter_context(tc.tile_pool(name="xw_sb", bufs=2))
        gath = ctx.enter_context(tc.tile_pool(name="gath", bufs=2))
        s_pool = ctx.enter_context(tc.tile_pool(name="s_pool", bufs=4))
        rhs_pool = ctx.enter_context(tc.tile_pool(name="rhs_pool", bufs=4))
        psum2 = ctx.enter_context(tc.tile_pool(name="psum2", bufs=2, space="PSUM"))
        out_sb = ctx.enter_context(tc.tile_pool(name="out_sb", bufs=4))

        # resident constants
        w_t = []
        for k in range(n_kc):
            wt = const.tile([128, f_out], bf16, tag=f"w{k}")
            nc.sync.dma_start(out=wt[:], in_=w_in[k * 128 : (k + 1) * 128, :])
            w_t.append(wt)
        meta_t = const.tile([128, 2 * J + GB * WIN], f32, tag="meta")
        nc.sync.dma_start(out=meta_t[:], in_=meta[:, :])
        cols_t = []
        for s in range(NSPLIT):
            ct = const.tile([128, Js[s] * 8], i16, tag=f"c{s}")
            nc.sync.dma_start(out=ct[:], in_=cols_d[s][:, :])
            cols_t.append(ct)

        # ---- phase 1: xw = (X @ W) in bf16, partition-major, 128-padded ----
        xw_pm = xw[:, :].rearrange("(p t) f -> p (t f)", p=128)  # [128, NT*128]
        stg = None
        g0 = 0
        s0 = 0
        xts = []
        for nt_i in range(NT):
            n0 = nt_i * 128
            m = min(128, n_nodes - n0)
            if nt_i % (SLAB // 128) == 0:
                s0 = n0
                sl = min(SLAB, n_nodes - s0)
                xts = []
                for k in range(n_kc):
                    xtk = xt_pool.tile([128, SLAB], bf16, tag=f"xt{k}")
                    nc.sync.dma_start(
                        out=xtk[:, :sl],
                        in_=xt[k * 128 : (k + 1) * 128, s0 : s0 + sl],
                    )
                    xts.append(xtk)
            if nt_i % GRP == 0:
                g0 = nt_i
                stg = xw_sb.tile([128, GRP * 128], bf16, tag="stg")
                if SIM_MEMSET:  # garbage bytes are never consumed on HW
                    nc.gpsimd.memset(stg[:], 0)
            ps = psum1.tile([128, f_out], f32, tag="ps1")
            for k in range(n_kc):
                nc.tensor.matmul(
                    out=ps[:m, :],
                    lhsT=xts[k][:, n0 - s0 : n0 - s0 + m],
                    rhs=w_t[k][:],
                    start=(k == 0),
                    stop=(k == n_kc - 1),
                )
            loc = nt_i - g0
            nc.vector.tensor_copy(
                out=stg[:m, loc * 128 : loc * 128 + f_out], in_=ps[:m, :]
            )
            if nt_i == NT - 1 or (nt_i + 1) % GRP == 0:
                gn = nt_i + 1 - g0
                nc.sync.dma_start(
                    out=xw_pm[:, g0 * 128 : (g0 + gn) * 128],
                    in_=stg[:, : gn * 128],
                )

        # ---- phase 2: dma_gather + batched one-hot matmul segment-sum ----
        # meta column of a (window, split, k) tile in consumption order
        def meta_col(w, s, k):
            return (
                sum(int(starts[s2][w]) for s2 in range(NSPLIT))
                + sum(int(T[s2][w]) for s2 in range(s))
                + k
            )

        # stream-position -> meta column (needed for batched builds)
        m_of = [np.zeros(Js[s], dtype=np.int64) for s in range(NSPLIT)]
        for w in range(n_win):
            for s in range(NSPLIT):
                for k in range(int(T[s][w])):
                    m_of[s][int(starts[s][w]) + k] = meta_col(w, s, k)

        chunks = {}  # (s, chunk_idx) -> gather tile
        batches = {}  # (s, batch_idx) -> (S_b, rhs_b)

        def ensure_chunk(s, tile_idx):
            ci = tile_idx // CH
            key = (s, ci)
            if key in chunks:
                return chunks[key]
            lo, hi_r = ranges[s]
            cn = min(CH, Js[s] - ci * CH)
            g = gath.tile([128, CH, 128], bf16, tag=f"g{s}")
            nc.gpsimd.dma_gather(
                out_ap=g[:, :cn, :],
                in_ap=xw[lo:hi_r, :],
                idxs_ap=cols_t[s][:, ci * CH * 8 : (ci * CH + cn) * 8],
                num_idxs=cn * 128,
                num_idxs_reg=cn * 128,
                elem_size=128,
                single_packet=False,
                # one SWDGE queue (= Q7 core pair) per source split: the
                # three desc-gen streams overlap.
                queue_num=s,
            )
            chunks[key] = g
            return g

        def ensure_batch(s, tile_idx):
            bi = tile_idx // GB
            key = (s, bi)
            if key in batches:
                return batches[key]
            b0 = bi * GB
            bn = min(GB, Js[s] - b0)
            g = ensure_chunk(s, b0)
            gs = b0 - (b0 // CH) * CH  # batch offset within its chunk
            # meta columns of this batch are contiguous within a (window,
            # split) group but the batch may span groups; split into
            # contiguous runs.
            cols = m_of[s][b0 : b0 + bn]
            S_b = s_pool.tile([128, GB, WIN], bf16, tag=f"S{s}")
            rhs_b = rhs_pool.tile([128, GB, f_out], bf16, tag=f"r{s}")
            runs = []
            r0 = 0
            for i in range(1, bn + 1):
                if i == bn or cols[i] != cols[i - 1] + 1:
                    runs.append((r0, i))
                    r0 = i
            for a, b in runs:
                n = b - a
                mc = int(cols[a])
                nc.vector.tensor_tensor(
                    out=S_b[:, a:b, :],
                    in0=meta_t[:, 2 * J : 2 * J + n * WIN].rearrange(
                        "p (b r) -> p b r", r=WIN
                    ),
                    in1=meta_t[:, mc : mc + n].to_broadcast([128, n, WIN]),
                    op=mybir.AluOpType.is_equal,
                )
                nc.vector.tensor_tensor(
                    out=rhs_b[:, a:b, :],
                    in0=g[:, gs + a : gs + b, 0:f_out],
                    in1=meta_t[:, J + mc : J + mc + n].to_broadcast(
                        [128, n, f_out]
                    ),
                    op=mybir.AluOpType.mult,
                )
            batches[key] = (S_b, rhs_b)
            return batches[key]

        for w in range(n_win):
            cur_ps = psum2.tile([128, f_out], f32, tag="ps2")
            n_t = int(sum(T[s][w] for s in range(NSPLIT)))
            ti = 0  # tile index within this window's consumption order
            for s in range(NSPLIT):
                for k in range(int(T[s][w])):
                    t_s = int(starts[s][w]) + k  # stream position
                    S_b, rhs_b = ensure_batch(s, t_s)
                    sl = t_s % GB
                    nc.tensor.matmul(
                        out=cur_ps[:],
                        lhsT=S_b[:, sl : sl + 1, :],
                        rhs=rhs_b[:, sl : sl + 1, :],
                        start=(ti == 0),
                        stop=(ti == n_t - 1),
                    )
                    ti += 1
            rows = min(WIN, shard - w * WIN)
            ot = out_sb.tile([128, f_out], f32, tag="ot")
            nc.vector.tensor_copy(out=ot[:rows, :], in_=cur_ps[:rows, :])
            nc.sync.dma_start(out=out[w * WIN : w * WIN + rows, :], in_=ot[:rows, :])
    nc.compile()
    return nc


def kernel(X, W, edge_row, edge_col, edge_vals):
    global LAST_RESULTS
    X = np.asarray(X, dtype=np.float32)
    W = np.asarray(W, dtype=np.float32)
    edge_row = np.asarray(edge_row, dtype=np.int32)
    edge_col = np.asarray(edge_col, dtype=np.int32)
    edge_vals = np.asarray(edge_vals, dtype=np.float32)

    in_maps, T = _prep(X, W, edge_row, edge_col, edge_vals)
    nc = _build_nc(T)
    trace = TRACE and _install_ntff_hook()
    res = run_bass_kernel_spmd(
        nc, in_maps, core_ids=list(range(N_CORES)), trace=trace
    )
    LAST_RESULTS = res
    out = np.concatenate([res.results[p]["out"] for p in range(N_CORES)], axis=0)
    return out.astype(np.float32)
